# revision 16
# baseline (speedup 1.0000x reference)
"""AGNN propagation kernel for 8 TRN2 NeuronCores.

Algorithm (matches reference):
    x_norm = x * rsqrt(sum(x^2, -1) + 1e-8)
    logit_e = beta * <x_norm[dst_e], x_norm[src_e]>        (in [-beta, beta])
    alpha_e = exp(logit_e) / (segsum_dst(exp(logit)) + 1e-8)
    out_i   = sum_{e: dst_e = i} alpha_e * x[src_e]

Because |logit| <= beta < 1, the segment-max subtraction in the reference is
numerically unnecessary (exp stays in [e^-1, e]); plain exp matches to ~1e-7.

Sharding: node-parallel, no collectives. Host sorts nodes by in-degree and
stripes them across the 8 cores (rank c, c+8, ...), so every core sees an
identical degree profile. Each core packs its nodes into blocks of 128
(1 node per SBUF partition); block b is padded to K_b = max degree in the
block (tight, because nodes are degree-sorted). A single indirect DMA per
block-group gathers the packed [x_norm | ||x||] rows of every edge source
into [128 nodes, K, 36] tiles; the segment softmax + weighted aggregation
are then per-partition free-dim reduces. Pad edge slots point at an
all-zero dummy row and are masked out of the softmax denominator.

Device phases (per core, one SPMD graph):
  prep: stream x -> packed[n] = [x_norm(32) | sqrt(ss+eps)(1) | pad(3)]
  main: per block: indirect-gather src rows, DVE dot products vs the
        block's own x_norm rows (per-partition broadcast), ScalarE exp,
        mask pads, free-dim reduces, divide, write dense output rows.
Host reassembles: out[node_order] = dense rows.
"""

import os
import numpy as np

import concourse.bass as bass
import concourse.bacc as bacc
import concourse.mybir as mybir
import concourse.tile as tile

F32 = mybir.dt.float32
I32 = mybir.dt.int32

N_CORES = 8
LAST_RESULT = None  # set by kernel() for profiling harnesses
P = 128          # SBUF partitions (= nodes per block)
EPS = 1e-8
PREP_ROWS = 32   # node rows per partition per prep supertile
BLOCKS_PER_GATHER = 2


# ----------------------------------------------------------------------------
# Host-side planning (index manipulation only; no FLOPs on tensor data)
# ----------------------------------------------------------------------------

def build_plan(edge_index: np.ndarray, n_nodes: int):
    src = np.asarray(edge_index[0], dtype=np.int64)
    dst = np.asarray(edge_index[1], dtype=np.int64)
    n_edges = src.shape[0]

    deg = np.bincount(dst, minlength=n_nodes).astype(np.int64)

    # Global degree-descending node order, striped over cores.
    order = np.argsort(-deg, kind="stable")

    nodes_per_core = (n_nodes + N_CORES - 1) // N_CORES
    blocks = (nodes_per_core + P - 1) // P
    slots = blocks * P                      # padded nodes per core

    # CSR of incoming edges (sorted by dst).
    eorder = np.argsort(dst, kind="stable")
    src_sorted = src[eorder]
    starts = np.zeros(n_nodes + 1, dtype=np.int64)
    np.cumsum(deg, out=starts[1:])

    # Shared block K profile: block b holds global ranks [b*P*NC, (b+1)*P*NC).
    deg_ranked = deg[order]
    K = np.zeros(blocks, dtype=np.int64)
    for b in range(blocks):
        lo = b * P * N_CORES
        hi = min(lo + P * N_CORES, n_nodes)
        K[b] = max(1, int(deg_ranked[lo:hi].max()) if hi > lo else 1)
    offs = np.zeros(blocks + 1, dtype=np.int64)
    np.cumsum(K, out=offs[1:])
    totk = int(offs[-1])

    dummy = n_nodes  # row of zeros in the padded x

    # node_of[c, s]: global node id at core c, slot s (or -1 pad).
    ranks = np.arange(slots) * N_CORES  # slot -> global rank base
    node_of = np.full((N_CORES, slots), -1, dtype=np.int64)
    for c in range(N_CORES):
        r = ranks + c
        valid = r < n_nodes
        node_of[c, valid] = order[r[valid]]

    return dict(
        n_nodes=n_nodes, n_edges=n_edges, blocks=blocks, slots=slots,
        K=K, offs=offs, totk=totk, dummy=dummy, node_of=node_of,
    )


# ----------------------------------------------------------------------------
# Bass graph builder (one SPMD graph shared by all cores)
# ----------------------------------------------------------------------------

def build_kernel(n_nodes_pad: int, d_feat: int, blocks: int, K, offs, totk: int,
                 prep_rows: int = PREP_ROWS):
    """n_nodes_pad: padded row count of x input (multiple of P*prep_rows)."""
    assert n_nodes_pad % (P * prep_rows) == 0
    supertiles = n_nodes_pad // (P * prep_rows)
    D = d_feat
    DP = D + 4   # packed row: [x_norm(D) | w(1) | pad(3)] -> 16B aligned for D=32
    kmax = int(max(K))
    assert kmax <= 512

    nc = bacc.Bacc(None, target_bir_lowering=False, debug=False)

    x_ext = nc.declare_dram_parameter("x", [n_nodes_pad, D], F32, isOutput=False)
    idx_ext = nc.declare_dram_parameter("idx", [P, totk], I32, isOutput=False)
    perm_ext = nc.declare_dram_parameter("perm", [P, blocks], I32, isOutput=False)
    deg_ext = nc.declare_dram_parameter("deg", [P, blocks], F32, isOutput=False)
    beta_ext = nc.declare_dram_parameter("beta", [P, 1], F32, isOutput=False)
    out_ext = nc.declare_dram_parameter("out", [blocks * P, D], F32, isOutput=True)

    groups = [list(range(g, min(g + BLOCKS_PER_GATHER, blocks)))
              for g in range(0, blocks, BLOCKS_PER_GATHER)]

    with tile.TileContext(nc) as tc:
        with (
            tc.tile_pool(name="dram", bufs=1, space="DRAM") as dram,
            tc.tile_pool(name="persist", bufs=1) as persist,
            tc.tile_pool(name="xin", bufs=3) as xin_pool,
            tc.tile_pool(name="pk", bufs=3) as pk_pool,
            tc.tile_pool(name="pscr", bufs=2) as pscr_pool,
            tc.tile_pool(name="gath", bufs=4) as gath_pool,
            tc.tile_pool(name="scr", bufs=3) as scr_pool,
            tc.tile_pool(name="sm", bufs=4) as sm_pool,
            tc.tile_pool(name="outp", bufs=3) as out_pool,
        ):
            packed = dram.tile([n_nodes_pad, DP], F32)

            # ---- persistent small tiles -------------------------------------
            beta_sb = persist.tile([P, 1], F32)
            nc.sync.dma_start(out=beta_sb[:], in_=beta_ext[:, :])
            idx_sb = persist.tile([P, totk], I32)
            nc.sync.dma_start(out=idx_sb[:], in_=idx_ext[:, :])
            deg_sb = persist.tile([P, blocks], F32)
            nc.sync.dma_start(out=deg_sb[:], in_=deg_ext[:, :])
            perm_sb = persist.tile([P, blocks], I32)
            nc.sync.dma_start(out=perm_sb[:], in_=perm_ext[:, :])
            iota_i = persist.tile([P, kmax], I32)
            nc.gpsimd.iota(iota_i[:], pattern=[[1, kmax]], base=0,
                           channel_multiplier=0)
            iota_f = persist.tile([P, kmax], F32)
            nc.vector.tensor_copy(iota_f[:], iota_i[:])
            eps_sb = persist.tile([P, 1], F32)
            nc.vector.memset(eps_sb[:], EPS)

            # ---- prep: packed rows [x_norm | w] -----------------------------
            x_r = x_ext[:, :].rearrange("(s p a) d -> s p a d", p=P, a=prep_rows)
            pk_r = packed[:].rearrange("(s p a) e -> s p a e", p=P, a=prep_rows)
            A = prep_rows
            for s in range(supertiles):
                xt = xin_pool.tile([P, A, D], F32)
                nc.sync.dma_start(out=xt[:], in_=x_r[s])
                sq = pscr_pool.tile([P, A, D], F32)
                nc.scalar.activation(sq[:], xt[:],
                                     mybir.ActivationFunctionType.Square)
                ss = sm_pool.tile([P, A], F32, tag="prep_ss")
                nc.vector.tensor_reduce(out=ss[:], in_=sq[:],
                                        axis=mybir.AxisListType.X,
                                        op=mybir.AluOpType.add)
                pk = pk_pool.tile([P, A, DP], F32)
                nc.scalar.activation(pk[:, :, D], ss[:],
                                     mybir.ActivationFunctionType.Sqrt,
                                     bias=eps_sb[:, :1])
                winv = sm_pool.tile([P, A], F32, tag="prep_winv")
                nc.vector.reciprocal(winv[:], pk[:, :, D])
                nc.vector.tensor_tensor(
                    out=pk[:, :, 0:D], in0=xt[:],
                    in1=winv[:, :, None].to_broadcast([P, A, D]),
                    op=mybir.AluOpType.mult)
                nc.vector.memset(pk[:, :, D + 1:DP], 0.0)
                nc.sync.dma_start(out=pk_r[s], in_=pk[:])

            # ---- gather the block-node (dst) packed rows --------------------
            # NOTE: this toolchain's indirect DMA only supports one descriptor
            # per partition per instruction (idx [P,1] -> dest [P, DP]).
            xnd_all = persist.tile([P, blocks, DP], F32)
            for b in range(blocks):
                nc.gpsimd.indirect_dma_start(
                    out=xnd_all[:, b, :], out_offset=None,
                    in_=packed[:],
                    in_offset=bass.IndirectOffsetOnAxis(
                        ap=perm_sb[:, b:b + 1], axis=0),
                )

            out_r = out_ext[:, :].rearrange("(b p) d -> b p d", p=P)

            # ---- main loop --------------------------------------------------
            for grp in groups:
                g0, g1 = grp[0], grp[-1]
                o0, o1 = int(offs[g0]), int(offs[g1 + 1])
                tk = o1 - o0

                gt = gath_pool.tile([P, tk, DP], F32, tag="gath")
                for c in range(tk):
                    nc.gpsimd.indirect_dma_start(
                        out=gt[:, c, :], out_offset=None,
                        in_=packed[:],
                        in_offset=bass.IndirectOffsetOnAxis(
                            ap=idx_sb[:, o0 + c:o0 + c + 1], axis=0),
                    )

                for b in grp:
                    kb = int(K[b])
                    oo = int(offs[b]) - o0
                    xs = gt[:, oo:oo + kb, 0:D]            # [P, kb, D]
                    wsrc = gt[:, oo:oo + kb, D]            # [P, kb]
                    xnd = xnd_all[:, b:b + 1, 0:D]         # [P, 1, D]

                    t = scr_pool.tile([P, kmax, D], F32, tag="tscr")
                    nc.vector.tensor_tensor(
                        out=t[:, :kb, :], in0=xs,
                        in1=xnd.to_broadcast([P, kb, D]),
                        op=mybir.AluOpType.mult)
                    d0 = sm_pool.tile([P, kmax], F32, tag="d0")
                    nc.vector.tensor_reduce(out=d0[:, :kb], in_=t[:, :kb, :],
                                            axis=mybir.AxisListType.X,
                                            op=mybir.AluOpType.add)
                    z = sm_pool.tile([P, kmax], F32, tag="z")
                    nc.scalar.activation(z[:, :kb], d0[:, :kb],
                                         mybir.ActivationFunctionType.Exp,
                                         scale=beta_sb[:, :])
                    # mask pad slots: z *= (deg > k)
                    v = sm_pool.tile([P, kmax], F32, tag="v")
                    nc.vector.tensor_tensor(
                        out=v[:, :kb],
                        in0=deg_sb[:, b:b + 1].to_broadcast([P, kb]),
                        in1=iota_f[:, :kb],
                        op=mybir.AluOpType.is_gt)
                    nc.vector.tensor_tensor(out=z[:, :kb], in0=z[:, :kb],
                                            in1=v[:, :kb],
                                            op=mybir.AluOpType.mult)
                    seg = sm_pool.tile([P, 1], F32, tag="seg")
                    nc.vector.tensor_reduce(out=seg[:], in_=z[:, :kb],
                                            axis=mybir.AxisListType.X,
                                            op=mybir.AluOpType.add)
                    nc.vector.tensor_scalar_add(seg[:], seg[:], EPS)
                    rec = sm_pool.tile([P, 1], F32, tag="rec")
                    nc.vector.reciprocal(rec[:], seg[:])
                    # z' = z * w_src ; payload = z' * x_norm_src
                    nc.vector.tensor_tensor(out=z[:, :kb], in0=z[:, :kb],
                                            in1=wsrc,
                                            op=mybir.AluOpType.mult)
                    nc.vector.tensor_tensor(
                        out=t[:, :kb, :], in0=xs,
                        in1=z[:, :kb, None].to_broadcast([P, kb, D]),
                        op=mybir.AluOpType.mult)
                    ob = out_pool.tile([P, D], F32, tag="ob")
                    nc.vector.tensor_reduce(
                        out=ob[:], in_=t[:, :kb, :].rearrange("p k d -> p d k"),
                        axis=mybir.AxisListType.X,
                        op=mybir.AluOpType.add)
                    nc.vector.tensor_scalar_mul(ob[:], ob[:], rec[:, :1])
                    nc.sync.dma_start(out=out_r[b], in_=ob[:])

    return nc


# ----------------------------------------------------------------------------
# Public entry point
# ----------------------------------------------------------------------------

def _pad_rows(n_rows: int, quantum: int) -> int:
    return ((n_rows + quantum - 1) // quantum) * quantum


def kernel(x: np.ndarray, beta: np.ndarray, edge_index: np.ndarray,
           _debug_sim: bool = False) -> np.ndarray:
    x = np.asarray(x, dtype=np.float32)
    beta = np.asarray(beta, dtype=np.float32)
    edge_index = np.asarray(edge_index)
    n_nodes, d_feat = x.shape

    plan = build_plan(edge_index, n_nodes)
    blocks, slots = plan["blocks"], plan["slots"]

    n_pad = _pad_rows(n_nodes + 1, P * PREP_ROWS)
    x_pad = np.zeros((n_pad, d_feat), dtype=np.float32)
    x_pad[:n_nodes] = x

    nc = build_kernel(n_pad, d_feat, blocks, plan["K"], plan["offs"],
                      plan["totk"])
    if not nc.is_finalized():
        nc.finalize()

    # per-core input maps
    src = np.asarray(edge_index[0], dtype=np.int64)
    dst = np.asarray(edge_index[1], dtype=np.int64)
    plan2 = _build_core_arrays(plan, src, dst, n_nodes)
    in_maps = []
    beta_b = np.broadcast_to(beta.reshape(1, 1), (P, 1)).astype(np.float32).copy()
    for c in range(N_CORES):
        in_maps.append({
            "x": x_pad,
            "idx": plan2["idx_all"][c],
            "perm": plan2["perm"][c],
            "deg": plan2["degm"][c],
            "beta": beta_b,
        })

    if _debug_sim:
        from concourse import bass_interp
        sim = bass_interp.MultiCoreSim(nc, N_CORES)
        for c in range(N_CORES):
            for k, vv in in_maps[c].items():
                sim.cores[c].tensor(k)[:] = vv
        sim.simulate()
        results = [{"out": sim.cores[c].mem_tensor("out").copy()}
                   for c in range(N_CORES)]
    else:
        from concourse.bass_utils import run_bass_kernel_spmd
        trace = bool(int(os.environ.get("AGNN_TRACE", "0")))
        tmpdir = os.environ.get("AGNN_TRACE_DIR") or None
        res = run_bass_kernel_spmd(nc, in_maps, core_ids=list(range(N_CORES)),
                                   trace=trace, tmpdir=tmpdir)
        results = res.results
        global LAST_RESULT
        LAST_RESULT = res

    out = np.zeros((n_nodes, d_feat), dtype=np.float32)
    node_of = plan["node_of"]
    for c in range(N_CORES):
        nd = node_of[c]
        valid = nd >= 0
        out[nd[valid]] = results[c]["out"][:slots][valid]
    return out


def _build_core_arrays(plan, src, dst, n_nodes):
    """Recompute the per-core index arrays (kept out of plan for clarity)."""
    deg = np.bincount(dst, minlength=n_nodes).astype(np.int64)
    eorder = np.argsort(dst, kind="stable")
    src_sorted = src[eorder]
    starts = np.zeros(n_nodes + 1, dtype=np.int64)
    np.cumsum(deg, out=starts[1:])

    blocks, totk = plan["blocks"], plan["totk"]
    K, offs, dummy = plan["K"], plan["offs"], plan["dummy"]
    node_of = plan["node_of"]

    idx_all = np.full((N_CORES, P, totk), dummy, dtype=np.int32)
    perm = np.full((N_CORES, P, blocks), dummy, dtype=np.int32)
    degm = np.zeros((N_CORES, P, blocks), dtype=np.float32)
    for c in range(N_CORES):
        for b in range(blocks):
            kb = int(K[b])
            nd = node_of[c, b * P:(b + 1) * P]
            valid = nd >= 0
            ndv = np.where(valid, nd, 0)
            d = np.where(valid, deg[ndv], 0)
            perm[c, :, b] = np.where(valid, nd, dummy)
            degm[c, :, b] = d.astype(np.float32)
            kk = np.arange(kb)[None, :]
            take = kk < d[:, None]
            pos = np.where(take, starts[ndv][:, None] + kk, 0)
            idx_all[c, :, offs[b]:offs[b] + kb] = np.where(
                take, src_sorted[pos], dummy)
    return dict(idx_all=idx_all, perm=perm, degm=degm)
